# revision 32
# baseline (speedup 1.0000x reference)
"""Trainium2 Bass kernel for DariushMultiHeadAttention (GQA + RoPE, causal).

Reference, for x [1, 2048, 1024]:
    q = (x @ Wq).reshape(S, 16, 64); k,v likewise with 4 kv heads
    q, k = rope(q), rope(k)
    causal softmax(q k^T / 8) @ v, concat heads, @ Wo + bo

Sharding: tensor-parallel over heads across the 8 cores. Core c owns
q heads {2c, 2c+1} and kv head c//2 (both q heads of a core share one
kv head: GQA group size is 4). Each core computes a full [2048, 1024]
partial of the output projection; the host sums the 8 partials (the
TP all-reduce) and adds bo. bq/bk/bv are zeros per the spec.

v5 design notes (schedule follows v2's consolidated phases — the PE
downclocks when stalls are sprinkled through the stream, so keep long
uninterrupted matmul runs; measured v3/v4 interleaved variants were
20us slower):
  - All matmul operands bf16 (1 cycle/row on the PE); PSUM f32.
  - Host pre-rearranges wq/wkv to [128, ec*128]: contiguous 2KB
    descriptors instead of 1024 x 256B. Input DMA dispatch is spread
    over the sync/scalar/gpsimd queues.
  - Phase A: kv+q projections interleaved per embedding chunk (8
    matmuls per xt chunk paces the incoming DMA).
  - Phase B: PSUM->SBUF bf16 casts of kv/qt on the Act engine; rope
    rotation matmuls + v transposes on the PE; rope combines as DVE
    multiplies + GpSimd adds, sequenced k0,q0,v0..3,k1,q1,v4..7,...
    so the first attention block's dependencies complete first.
  - Phase C per q block: attention h0, h1, then output projection.
    Scores in [k, q] orientation; exp on Act over k-chunk PAIRS fused
    in one [128, <=1024] 2-bank PSUM tile. The Act attention stream
    is pure exp (copies live on DVE) since Act throughput
    ((N+352)/1.2ns) is the attention-phase critical path. Causal mask
    = bf16 triangular multiply on GpSimd, diagonal chunks only.
  - PV stationary is [ones*64 | v]: rows 0:63 of the PV output get
    the softmax denominator broadcast across 64 partitions for free;
    rows 64:127 are o. Denominator sits at partition base 0 because
    reciprocal_approx_fast (custom DVE op) misreads at a base-64
    offset on HW (verified). Normalize = recip_approx_fast + one DVE
    multiply writing on2 bf16.
  - Output projection packs both heads (on2 [128, S]) in K=128
    matmuls; the two [128,512] PSUM->SBUF copies per chunk go to DVE,
    and the two y half-DMAs dispatch from gpsimd and sync so neither
    serial wait chain gates ysb recycling.
"""
import sys

if "/opt/trn_rl_repo" not in sys.path:
    sys.path.insert(0, "/opt/trn_rl_repo")

import numpy as np

S = 2048
EMB = 1024
D = 64
NQ = 16
NKV = 4
NCORES = 8
ROPE_BASE = 10000.0
SCALE = 1.0 / 8.0

SC = S // 128   # 16 sequence chunks
EC = EMB // 128  # 8 embedding (contraction) chunks
QB = S // 512   # 4 q blocks

_CACHE = {}


def _build_nc(dbg=False, sim_safe=False):
    import concourse.bacc as bacc
    import concourse.mybir as mybir
    import concourse.tile as tile

    f32 = mybir.dt.float32
    bf16 = mybir.dt.bfloat16

    nc = bacc.Bacc("TRN2", target_bir_lowering=False, debug=False)

    xt_d = nc.dram_tensor("xt", [EMB, S], bf16, kind="ExternalInput")
    wq_d = nc.dram_tensor("wq", [128, EC * 128], bf16, kind="ExternalInput")
    wkv_d = nc.dram_tensor("wkv", [128, EC * 128], bf16, kind="ExternalInput")
    woab_d = nc.dram_tensor("woab", [128, EMB], bf16, kind="ExternalInput")
    cos_d = nc.dram_tensor("cos", [128, S], f32, kind="ExternalInput")
    sin_d = nc.dram_tensor("sin", [128, S], f32, kind="ExternalInput")
    rot_d = nc.dram_tensor("rot", [128, 128], bf16, kind="ExternalInput")
    dup_d = nc.dram_tensor("dup", [D, 128], bf16, kind="ExternalInput")
    rotdup_d = nc.dram_tensor("rotdup", [D, 128], bf16, kind="ExternalInput")
    tri_d = nc.dram_tensor("tri", [128, 128], bf16, kind="ExternalInput")
    idt_d = nc.dram_tensor("idt", [128, D], bf16, kind="ExternalInput")
    y_d = nc.dram_tensor("y", [S, EMB], bf16, kind="ExternalOutput")
    dbg_d = {}
    if dbg:
        for nm, shp, dt_ in [
            ("kv", [128, S], bf16), ("qt", [128, S], bf16),
            ("krope2", [128, S], bf16), ("qrope", [128, S], bf16),
            ("vsb", [128, SC * 128], bf16), ("on2", [128, S], bf16),
            ("pso00", [128, 512], f32), ("rec00", [64, 512], f32),
        ]:
            dbg_d[nm] = nc.dram_tensor("dbg_" + nm, shp, dt_,
                                       kind="ExternalOutput")

    with tile.TileContext(nc) as tc:
        with tc.tile_pool(name="const", bufs=1) as cpool, \
             tc.tile_pool(name="big", bufs=1) as big, \
             tc.tile_pool(name="tmp", bufs=3) as tmp, \
             tc.tile_pool(name="wtp", bufs=6) as wtp, \
             tc.tile_pool(name="ypool", bufs=4) as ypool, \
             tc.tile_pool(name="psS", bufs=2, space="PSUM") as psS, \
             tc.tile_pool(name="psP", bufs=4, space="PSUM") as psP:

            # ---- input DMAs, dispatch spread over 3 queues ----
            wkv_sb = cpool.tile([128, EC, 128], bf16, name="wkv_sb")
            nc.sync.dma_start(
                out=wkv_sb, in_=wkv_d.rearrange("p (ec m) -> p ec m", m=128))
            xts = []
            xt_t = cpool.tile([128, S], bf16, name="xt0", tag="xt0")
            nc.sync.dma_start(out=xt_t, in_=xt_d[0:128, :])
            xts.append(xt_t)
            wq_sb = cpool.tile([128, EC, 128], bf16, name="wq_sb")
            nc.scalar.dma_start(
                out=wq_sb, in_=wq_d.rearrange("p (ec m) -> p ec m", m=128))
            for ec in range(1, EC):
                xt_t = cpool.tile([128, S], bf16, name=f"xt{ec}", tag=f"xt{ec}")
                eng = [nc.sync, nc.scalar, nc.gpsimd][ec % 3]
                eng.dma_start(out=xt_t, in_=xt_d[ec * 128:(ec + 1) * 128, :])
                xts.append(xt_t)
            dup_sb = cpool.tile([D, 128], bf16, name="dup_sb")
            nc.gpsimd.dma_start(out=dup_sb, in_=dup_d[:, :])
            rotdup_sb = cpool.tile([D, 128], bf16, name="rotdup_sb")
            nc.gpsimd.dma_start(out=rotdup_sb, in_=rotdup_d[:, :])
            rot_sb = cpool.tile([128, 128], bf16, name="rot_sb")
            nc.gpsimd.dma_start(out=rot_sb, in_=rot_d[:, :])
            cos_sb = cpool.tile([128, S], f32, name="cos_sb")
            nc.scalar.dma_start(out=cos_sb, in_=cos_d[:, :])
            sin_sb = cpool.tile([128, S], f32, name="sin_sb")
            nc.sync.dma_start(out=sin_sb, in_=sin_d[:, :])
            idt_sb = cpool.tile([128, D], bf16, name="idt_sb")
            nc.gpsimd.dma_start(out=idt_sb, in_=idt_d[:, :])
            tri_sb = cpool.tile([128, 128], bf16, name="tri_sb")
            nc.gpsimd.dma_start(out=tri_sb, in_=tri_d[:, :])
            woab_sb = cpool.tile([128, EMB], bf16, name="woab_sb")
            nc.gpsimd.dma_start(out=woab_sb, in_=woab_d[:, :])

            # ---- persistent activations (all bf16) ----
            kv_sb = big.tile([128, S], bf16, name="kv_sb")    # [k^T; v^T]
            qt_sb = big.tile([128, S], bf16, name="qt_sb")    # q^T pre-rope
            krope2 = big.tile([128, S], bf16, name="krope2")  # rope(k)^T x2
            qrope = big.tile([128, S], bf16, name="qrope")    # q^T post-rope
            v_sb = big.tile([128, SC, 128], bf16, name="v_sb")  # ones*64 | v
            on2 = big.tile([128, S], bf16, name="on2")        # o^T both heads

            nc.gpsimd.memset(v_sb[:, :, 0:D], 1.0)

            # ---- phase A: kv + q projections, ec-outer interleaved ----
            kv_ps = [psS.tile([128, 1024], f32, name=f"pskv{i}", tag="psS")
                     for i in range(2)]
            q_ps = [psP.tile([128, 512], f32, name=f"psq{i}", tag="psP")
                    for i in range(QB)]
            for ec in range(EC):
                st, sp = ec == 0, ec == EC - 1
                for qb in range(QB):
                    nc.tensor.matmul(
                        kv_ps[qb // 2][:, (qb % 2) * 512:(qb % 2) * 512 + 512],
                        wkv_sb[:, ec, :], xts[ec][:, qb * 512:qb * 512 + 512],
                        start=st, stop=sp,
                    )
                for qb in range(QB):
                    nc.tensor.matmul(
                        q_ps[qb], wq_sb[:, ec, :],
                        xts[ec][:, qb * 512:qb * 512 + 512],
                        start=st, stop=sp,
                    )

            # ---- phase B: drains ----
            # kv/qt casts on the Act engine (idle here; DVE does combines)
            for qb in range(QB):
                lo = qb * 512
                nc.scalar.copy(
                    kv_sb[:, lo:lo + 512],
                    kv_ps[qb // 2][:, (qb % 2) * 512:(qb % 2) * 512 + 512],
                )
            for qb in range(QB):
                lo = qb * 512
                nc.scalar.copy(qt_sb[:, lo:lo + 512], q_ps[qb])

            def rope_combine(ps_plain, ps_rot, out_ap, cos_ap, sin_ap, tag):
                t1 = tmp.tile([128, 512], f32, name=f"t1{tag}", tag="t1")
                nc.vector.tensor_tensor(t1, ps_plain, cos_ap, mybir.AluOpType.mult)
                t2 = tmp.tile([128, 512], f32, name=f"t2{tag}", tag="t2")
                nc.vector.tensor_tensor(t2, ps_rot, sin_ap, mybir.AluOpType.mult)
                nc.gpsimd.tensor_tensor(out_ap, t1, t2, mybir.AluOpType.add)

            def drain_k(qb):
                lo = qb * 512
                kkr = psS.tile([128, 1024], f32, name=f"kkr{qb}", tag="psS")
                nc.tensor.matmul(kkr[:, 0:512], dup_sb, kv_sb[0:D, lo:lo + 512],
                                 start=True, stop=True)
                nc.tensor.matmul(kkr[:, 512:1024], rotdup_sb,
                                 kv_sb[0:D, lo:lo + 512], start=True, stop=True)
                rope_combine(kkr[:, 0:512], kkr[:, 512:1024],
                             krope2[:, lo:lo + 512], cos_sb[:, lo:lo + 512],
                             sin_sb[:, lo:lo + 512], f"k{qb}")

            def drain_q(qb):
                lo = qb * 512
                rq = psS.tile([128, 1024], f32, name=f"rq{qb}", tag="psS")
                nc.tensor.matmul(rq[:, 0:512], rot_sb, qt_sb[:, lo:lo + 512],
                                 start=True, stop=True)
                rope_combine(q_ps[qb], rq[:, 0:512], qrope[:, lo:lo + 512],
                             cos_sb[:, lo:lo + 512], sin_sb[:, lo:lo + 512],
                             f"q{qb}")

            def vtrans(sc):
                ps_v = psS.tile([128, D], bf16, name=f"psv{sc}", tag="psS")
                nc.tensor.transpose(
                    ps_v, kv_sb[D:128, sc * 128:(sc + 1) * 128],
                    idt_sb[D:128, :])
                nc.vector.tensor_copy(v_sb[:, sc, D:128], ps_v)

            # ---- phase C: attention + output projection per q block ----
            def attn_task(h, qb):
                lo = qb * 512
                kc_max = 4 * (qb + 1)
                hp = h * 64
                pso = psP.tile([128, 512], f32, name=f"pso{h}_{qb}", tag="psP")
                npairs = kc_max // 2
                for p in range(npairs):
                    pss = psS.tile([128, 1024], f32, name=f"pss{h}_{qb}_{p}",
                                   tag="psS")
                    wt = wtp.tile([128, 1024], bf16, name=f"wt{h}_{qb}_{p}",
                                  tag="wt")
                    offs = []
                    for j, kc in enumerate((2 * p, 2 * p + 1)):
                        off = max(kc - 4 * qb, 0) * 128
                        n = 512 - off
                        offs.append((off, n))
                        nc.tensor.matmul(
                            pss[:, j * 512:j * 512 + n],
                            krope2[hp:hp + D, kc * 128:(kc + 1) * 128],
                            qrope[hp:hp + D, lo + off:lo + 512],
                            start=True, stop=True,
                        )
                    nA, nB = offs[0][1], offs[1][1]
                    if nA == 512:
                        nc.scalar.activation(
                            wt[:, 0:512 + nB], pss[:, 0:512 + nB],
                            mybir.ActivationFunctionType.Exp, scale=SCALE)
                    else:
                        nc.scalar.activation(
                            wt[:, 0:nA], pss[:, 0:nA],
                            mybir.ActivationFunctionType.Exp, scale=SCALE)
                        nc.scalar.activation(
                            wt[:, 512:512 + nB], pss[:, 512:512 + nB],
                            mybir.ActivationFunctionType.Exp, scale=SCALE)
                    for j, kc in enumerate((2 * p, 2 * p + 1)):
                        if kc - 4 * qb >= 0:
                            nc.gpsimd.tensor_tensor(
                                wt[:, j * 512:j * 512 + 128],
                                wt[:, j * 512:j * 512 + 128],
                                tri_sb, mybir.AluOpType.mult)
                    for j, kc in enumerate((2 * p, 2 * p + 1)):
                        off, n = offs[j]
                        nc.tensor.matmul(
                            pso[:, off:512],
                            v_sb[:, kc, :],
                            wt[:, j * 512:j * 512 + n],
                            start=(p == 0 and j == 0),
                            stop=(p == npairs - 1 and j == 1),
                        )
                # rows 0:64 hold the denominator broadcast across 64
                # partitions (ones columns); rows 64:128 hold o.
                if dbg and h == 0 and qb == 0:
                    pso_cp = tmp.tile([128, 512], f32, name="psocp", tag="psocp")
                    nc.vector.tensor_copy(pso_cp, pso)
                    nc.sync.dma_start(out=dbg_d["pso00"][:, :], in_=pso_cp)
                rec = tmp.tile([64, 512], f32, name=f"rec{h}_{qb}", tag="rec")
                nc.vector.reciprocal_approx_fast(out=rec, in_=pso[0:64, :])
                if dbg and h == 0 and qb == 0:
                    nc.sync.dma_start(out=dbg_d["rec00"][:, :], in_=rec)
                nc.vector.tensor_tensor(
                    on2[hp:hp + 64, lo:lo + 512], pso[64:128, :], rec,
                    mybir.AluOpType.mult)

            def yproj(sc, act_copy=False):
                ysb = ypool.tile([128, EMB], bf16, name=f"ysb{sc}", tag="ysb")
                for nb in range(2):
                    psy = psP.tile([128, 512], f32, name=f"psy{sc}_{nb}",
                                   tag="psP")
                    nc.tensor.matmul(
                        psy, on2[:, sc * 128:(sc + 1) * 128],
                        woab_sb[:, nb * 512:(nb + 1) * 512],
                        start=True, stop=True)
                    # copies stay off the Act engine while exps still flow;
                    # the final block splits copies DVE/Act to shorten the
                    # drain tail
                    if act_copy and nb == 1:
                        nc.scalar.copy(ysb[:, 512:1024], psy)
                    else:
                        nc.vector.tensor_copy(
                            ysb[:, nb * 512:(nb + 1) * 512], psy)
                    if nb == 0:
                        nc.gpsimd.dma_start(
                            out=y_d[sc * 128:(sc + 1) * 128, 0:512],
                            in_=ysb[:, 0:512])
                    else:
                        nc.sync.dma_start(
                            out=y_d[sc * 128:(sc + 1) * 128, 512:1024],
                            in_=ysb[:, 512:1024])

            # ---- consolidated schedule (interleaved variants measured
            # 10-16us slower: sprinkled stalls downclock the PE) ----
            for qb in range(QB):
                drain_k(qb)
                drain_q(qb)
                for sc in range(4 * qb, 4 * qb + 4):
                    vtrans(sc)
            for qb in range(QB):
                attn_task(0, qb)
                attn_task(1, qb)
                for sc in range(4 * qb, 4 * qb + 4):
                    # last block: no exps remain, so splitting its copies
                    # onto the idle Act engine shortens the drain tail
                    yproj(sc, act_copy=(qb == QB - 1))

            if dbg:
                nc.sync.dma_start(out=dbg_d["kv"][:, :], in_=kv_sb)
                nc.sync.dma_start(out=dbg_d["qt"][:, :], in_=qt_sb)
                nc.sync.dma_start(out=dbg_d["krope2"][:, :], in_=krope2)
                nc.sync.dma_start(out=dbg_d["qrope"][:, :], in_=qrope)
                nc.sync.dma_start(
                    out=dbg_d["vsb"][:, :],
                    in_=v_sb.rearrange("p a b -> p (a b)"))
                nc.sync.dma_start(out=dbg_d["on2"][:, :], in_=on2)

    nc.compile()
    return nc


def _rope_tables():
    inv_freq = 1.0 / (ROPE_BASE ** (np.arange(0, D, 2, dtype=np.float64) / D))
    pos = np.arange(S, dtype=np.float64)
    p = np.arange(128)
    ang = pos[None, :] * inv_freq[p % 32][:, None]  # [128, S]
    return np.cos(ang).astype(np.float32), np.sin(ang).astype(np.float32)


def _rot_single():
    rr = np.zeros((D, D), np.float32)
    for d in range(32):
        rr[d, d + 32] = -1.0  # rot(t)[d] = -t[d+32]
    for d in range(32, D):
        rr[d, d - 32] = 1.0   # rot(t)[d] = t[d-32]
    return rr


def _in_maps(x, Wq, Wk, Wv, Wo):
    import ml_dtypes

    bf = ml_dtypes.bfloat16
    xt = np.ascontiguousarray(x.reshape(S, EMB).T.astype(bf))
    cos_t, sin_t = _rope_tables()
    rr = _rot_single()
    rot = np.zeros((128, 128), np.float32)
    rot[0:D, 0:D] = rr.T
    rot[D:128, D:128] = rr.T
    dup = np.zeros((128, D), np.float32)   # Dup @ k duplicates k on both halves
    dup[0:D, 0:D] = np.eye(D)
    dup[D:128, 0:D] = np.eye(D)
    rot2 = np.zeros((128, 128), np.float32)
    rot2[0:D, 0:D] = rr
    rot2[D:128, D:128] = rr
    rotdup = rot2 @ dup                    # (R2 @ Dup) @ k
    tri = np.triu(np.ones((128, 128), np.float32))
    idt = np.concatenate([np.eye(D, dtype=np.float32)] * 2, axis=0)

    def warr(w):
        # [1024, 128] -> [128, ec*128] with w_r[p, ec*128+m] = w[ec*128+p, m]
        return np.ascontiguousarray(
            w.reshape(EC, 128, 128).transpose(1, 0, 2).reshape(128, EC * 128)
            .astype(bf))

    maps = []
    for c in range(NCORES):
        hk = c // 2
        maps.append({
            "xt": xt,
            "wq": warr(Wq[:, c * 128:(c + 1) * 128]),
            "wkv": warr(np.concatenate(
                [Wk[:, hk * D:(hk + 1) * D], Wv[:, hk * D:(hk + 1) * D]],
                axis=1)),
            "woab": np.ascontiguousarray(Wo[c * 128:(c + 1) * 128, :].astype(bf)),
            "cos": cos_t,
            "sin": sin_t,
            "rot": rot.astype(bf),
            "dup": np.ascontiguousarray(dup.T.astype(bf)),
            "rotdup": np.ascontiguousarray(rotdup.T.astype(bf)),
            "tri": tri.astype(bf),
            "idt": idt.astype(bf),
        })
    return maps


def _run(x, Wq, bq, Wk, bk, Wv, bv, Wo, bo, trace=False, trace_kwargs=None):
    from concourse import bass_utils

    if "nc" not in _CACHE:
        _CACHE["nc"] = _build_nc()
    nc = _CACHE["nc"]
    maps = _in_maps(
        np.asarray(x, np.float32), np.asarray(Wq, np.float32),
        np.asarray(Wk, np.float32), np.asarray(Wv, np.float32),
        np.asarray(Wo, np.float32),
    )
    res = bass_utils.run_bass_kernel_spmd(
        nc, maps, core_ids=list(range(NCORES)), trace=trace,
        **(trace_kwargs or {}),
    )
    y = np.zeros((S, EMB), np.float64)
    for c in range(NCORES):
        y += res.results[c]["y"].astype(np.float64)
    y += np.asarray(bo, np.float64)[None, :]
    return y.astype(np.float32).reshape(1, S, EMB), res


def kernel(x, Wq, bq, Wk, bk, Wv, bv, Wo, bo):
    out, _ = _run(x, Wq, bq, Wk, bk, Wv, bv, Wo, bo, trace=False)
    return out
